# revision 36
# baseline (speedup 1.0000x reference)
"""AttentionSuper (AutoFormer 2D rel-pos attention) Trainium2 Bass kernel.

B=64,N=197,C=640,H=10,D=64 ; data-parallel over batch: 8 batches/core x 8 cores.
Layout (per core, feature dim on partitions):
  xT (640,1640) -> qkT (10x[128,1640] tiles, tok free), v per batch (tok part)
  S^T[k,q] per (b,h) with rel-pos bias via constant one-hot matmuls.
  Rel-pos shift gathers are SBUF->SBUF DMAs with >=28B runs:
    - row (v-table) shifts act on the natural q order (row-groups contiguous)
    - col (h-table) shifts act on a grid-transposed (col-major) copy built by
      one strided vector-engine copy; the expansion matmul reads the col-major
      store back in natural order through a strided moving AP.
  Normalization: reciprocal of the rowsum row of the block-sum matmul,
  broadcast by a ones-matmul; applied to attn@v and to the block sums before
  the reverse (rel-pos-v) scatter, so pass 3 is one matmul + one in-place add.
Host: transposes/reshapes, builds constant tables, adds the cls-row rpv fix.
"""

import os
import numpy as np
import ml_dtypes

B, N, C = 64, 197, 640
H, D = 10, 64
MRP = 14
TABLE = 30
NB = 8          # batches per core
NCORES = 8
NT = NB * N     # 1576 tokens per core
NTP = NT + 64   # padded to 1640
NP = 200        # per-(b,h) q block width, natural order
NQ = 196        # per-(b,h) q block width, col-major (grid only)
HB = 2          # batches per gather group (quarter)
SCALE = D ** -0.5

_dt = None


def _consts():
    """Constant one-hot matrices (host, numpy)."""
    r = np.arange(N - 1)
    kr = r // 14
    kc = r % 14
    # E2 (197,80): attn^T -> block sums. col 0 ones (rowsum);
    # cols 1..14 v-blocks, 15 v-cls; cols 64..77 h-blocks, 78 h-cls
    E2 = np.zeros((N, 80), np.float32)
    E2[:, 0] = 1.0
    E2[1:, :][np.arange(N - 1), 1 + kr] = 1.0
    E2[0, 15] = 1.0
    E2[1:, :][np.arange(N - 1), 64 + kc] = 1.0
    E2[0, 78] = 1.0
    # EF2v (16,197): row 0 h-cls, rows 1..14 v-shift rows, 15 v-cls
    EF2v = np.zeros((16, N), np.float32)
    EF2v[0, 0] = 1.0
    EF2v[:, 1:][1 + kr, np.arange(N - 1)] = 1.0
    EF2v[15, 0] = 1.0
    # EF2h (14,197): rows j = [kc(k)=j], k>=1
    EF2h = np.zeros((14, N), np.float32)
    EF2h[:, 1:][kc, np.arange(N - 1)] = 1.0
    return E2, EF2v, EF2h


def _build_nc():
    import concourse.bass as bass  # noqa: F401
    import concourse.mybir as mybir
    from concourse import bacc
    from concourse.tile import TileContext

    global _dt
    _dt = mybir.dt
    f32 = mybir.dt.float32
    bf16 = mybir.dt.bfloat16
    EXP = mybir.ActivationFunctionType.Exp

    nc = bacc.Bacc("TRN2", target_bir_lowering=False, debug=False,
                   num_devices=NCORES)

    xT_d = nc.dram_tensor("xT", [C, NTP], bf16, kind="ExternalInput")
    wqkvT_d = nc.dram_tensor("wqkvT", [C, 3 * C], bf16, kind="ExternalInput")
    wprojT_d = nc.dram_tensor("wprojT", [C, C], bf16, kind="ExternalInput")
    bproj_d = nc.dram_tensor("bproj", [C, 1], f32, kind="ExternalInput")
    rpkT2_d = nc.dram_tensor("rpkT2", [128, 128], bf16, kind="ExternalInput")
    rpv2_d = nc.dram_tensor("rpv2", [2 * TABLE, D], bf16, kind="ExternalInput")
    EF2v_d = nc.dram_tensor("EF2v", [16, N], bf16, kind="ExternalInput")
    EF2h_d = nc.dram_tensor("EF2h", [14, N], bf16, kind="ExternalInput")
    E2_d = nc.dram_tensor("E2", [N, 80], bf16, kind="ExternalInput")
    yT_d = nc.dram_tensor("yT", [C, NTP], f32, kind="ExternalOutput")

    W = NP
    W2 = NQ
    NBLK = HB * H         # 40 (b,h) blocks per half

    from contextlib import ExitStack
    with TileContext(nc) as tc, ExitStack() as es:
        es.enter_context(nc.allow_low_precision(reason="bf16 attention kernel"))
        cp = es.enter_context(tc.tile_pool(name="const", bufs=1))
        qp = es.enter_context(tc.tile_pool(name="qk", bufs=1))
        op = es.enter_context(tc.tile_pool(name="outacc", bufs=1))
        vp = es.enter_context(tc.tile_pool(name="v", bufs=1))
        gp = es.enter_context(tc.tile_pool(name="gather", bufs=1))
        ap = es.enter_context(tc.tile_pool(name="attn", bufs=4))
        sp = es.enter_context(tc.tile_pool(name="small", bufs=3))
        pp = es.enter_context(tc.tile_pool(name="ps", bufs=2, space="PSUM"))
        pq = es.enter_context(tc.tile_pool(name="psq", bufs=3, space="PSUM"))
        p3 = es.enter_context(tc.tile_pool(name="ps3", bufs=3, space="PSUM"))

        # ---- load constants / weights ----
        wq = [cp.tile([128, 3 * C], bf16, tag=f"wq{c}", name=f"wq{c}") for c in range(5)]
        wp = [cp.tile([128, C], bf16, tag=f"wp{c}", name=f"wp{c}") for c in range(5)]
        xT = [cp.tile([128, NTP], bf16, tag=f"xT{c}", name=f"xT{c}") for c in range(5)]
        bpr = [cp.tile([128, 1], f32, tag=f"bp{c}", name=f"bpr{c}") for c in range(5)]
        for c in range(5):
            nc.sync.dma_start(out=wq[c][:], in_=wqkvT_d[128 * c:128 * (c + 1), :])
            nc.sync.dma_start(out=wp[c][:], in_=wprojT_d[128 * c:128 * (c + 1), :])
            nc.sync.dma_start(out=xT[c][:], in_=xT_d[128 * c:128 * (c + 1), :])
            nc.sync.dma_start(out=bpr[c][:], in_=bproj_d[128 * c:128 * (c + 1), :])
        rpkT2 = cp.tile([128, 128], bf16)
        rpv_v = cp.tile([TABLE, D], bf16)
        rpv_h = cp.tile([TABLE, D], bf16)
        EF2v = cp.tile([16, N], bf16)
        EF2h = cp.tile([14, N], bf16)
        E2a = cp.tile([128, 80], bf16)
        E2b = cp.tile([69, 80], bf16)
        nc.sync.dma_start(out=rpkT2[:], in_=rpkT2_d[:])
        nc.sync.dma_start(out=rpv_v[:], in_=rpv2_d[0:TABLE, :])
        nc.sync.dma_start(out=rpv_h[:], in_=rpv2_d[TABLE:2 * TABLE, :])
        nc.sync.dma_start(out=EF2v[:], in_=EF2v_d[:])
        nc.sync.dma_start(out=EF2h[:], in_=EF2h_d[:])
        nc.sync.dma_start(out=E2a[:], in_=E2_d[0:128, :])
        nc.sync.dma_start(out=E2b[:], in_=E2_d[128:197, :])

        # per-quarter gather tiles; Gv/Gh double-buffered via gp2
        gp2 = es.enter_context(tc.tile_pool(name="g2", bufs=2))
        svb_v = gp.tile([TABLE, NBLK * W], bf16, tag="svbv", name="svbv")
        svb_hT = gp.tile([TABLE, NBLK * W2], bf16, tag="svbhT", name="svbhT")
        bstb_v = gp.tile([16, NBLK * W], bf16, tag="bstbv", name="bstbv")
        bstb_hT = gp.tile([16, NBLK * W2], bf16, tag="bstbhT", name="bstbhT")
        C2v = gp.tile([TABLE, NBLK * W], bf16, tag="C2v", name="C2v")
        C2h = gp.tile([TABLE, NBLK * W2], bf16, tag="C2h", name="C2h")
        # rows 1,29 of C2v/C2h are read by the rpv matmuls but never written
        nc.gpsimd.memset(C2v[:], 0.0)
        nc.gpsimd.memset(C2h[:], 0.0)

        def blkv(bl, h):
            return (bl * H + h) * W

        WV = NBLK * W
        WH = NBLK * W2
        APc = type(svb_v[:])

        def mkap(tile, offset, dims):
            return APc(tile[:].tensor, offset, dims)

        # ---- qk projection: qkT[o, tok] for o in 0..1279 ----
        qkT = [qp.tile([128, NTP], bf16, tag=f"qkT{m}", name=f"qkT{m}") for m in range(10)]
        chunks = [(0, 512), (512, 512), (1024, 512), (1536, NTP - 1536)]
        for m in range(10):
            for (o0, on) in chunks:
                acc = pp.tile([128, on], f32, tag="projps")
                for c in range(5):
                    nc.tensor.matmul(
                        acc[:], wq[c][:, 128 * m:128 * (m + 1)],
                        xT[c][:, o0:o0 + on],
                        start=(c == 0), stop=(c == 4))
                nc.scalar.copy(out=qkT[m][:, o0:o0 + on], in_=acc[:])

        # ---- v projection for all batches: v[b] = (tok part, 640 free) ----
        vt = [[vp.tile([128, C], bf16, tag=f"v{b}_0", name=f"v{b}_0"),
               vp.tile([69, C], bf16, tag=f"v{b}_1", name=f"v{b}_1")]
              for b in range(NB)]
        for b in range(NB):
            t0 = b * N
            for s, (ts, tn) in enumerate([(t0, 128), (t0 + 128, 69)]):
                for (o0, on) in [(0, 512), (512, 128)]:
                    acc = pp.tile([tn, on], f32, tag="projps", name="accv")
                    for c in range(5):
                        nc.tensor.matmul(
                            acc[:], xT[c][:, ts:ts + tn],
                            wq[c][:, 2 * C + o0:2 * C + o0 + on],
                            start=(c == 0), stop=(c == 4))
                    nc.scalar.copy(out=vt[b][s][:, o0:o0 + on], in_=acc[:])

        # ---- out accumulator (c part, tok free) ----
        outT = [op.tile([128, NTP], bf16, tag=f"oT{m}", name=f"oT{m}") for m in range(5)]
        for m in range(5):
            nc.gpsimd.memset(outT[m][:, NT:NTP], 0.0)

        pending_pass3 = None

        def emit_pass1_and_G(bs):
            # pass 1: combined v+h table matmul; h-part written straight to
            # col-major svb_hT, h-cls row straight into Gv row 0
            Gv = gp2.tile([16, NBLK * W], bf16, tag="Gv", name="Gv")
            Gh = gp2.tile([14, NBLK * W2], bf16, tag="Gh", name="Gh")
            shT_b = svb_hT[:].rearrange("p (blk c r) -> p blk c r",
                                        blk=NBLK, c=14)
            for bl, b in enumerate(bs):
                t0 = b * N
                for hpi in range(H // 2):
                    be, bo = blkv(bl, 2 * hpi), blkv(bl, 2 * hpi + 1)
                    s_ps = p3.tile([128, W], f32, tag="sm", name="svps")
                    nc.tensor.matmul(s_ps[:], rpkT2[:],
                                     qkT[hpi][:, t0:t0 + W],
                                     start=True, stop=True)
                    nc.scalar.copy(out=svb_v[:, be:be + W], in_=s_ps[0:30, :])
                    nc.vector.tensor_copy(out=svb_v[:, bo:bo + W],
                                          in_=s_ps[64:94, :])
                    for p0, blk in ((32, 2 * hpi), (96, 2 * hpi + 1)):
                        i = bl * H + blk
                        nc.vector.tensor_copy(
                            out=shT_b[:, i:i + 1, :, :],
                            in_=s_ps[p0:p0 + 30, 1:197].rearrange(
                                "p (r c) -> p c r", r=14))
                        nc.vector.tensor_copy(
                            out=Gv[0:1, i * W + 1:i * W + 197],
                            in_=s_ps[p0:p0 + 1, 1:197])

            # gather Gv (natural) / Gh (col-major) via SBUF->SBUF DMAs
            Gv_q = Gv[:].rearrange("p (blk w) -> p blk w", blk=NBLK)
            sv_q = svb_v[:].rearrange("p (blk w) -> p blk w", blk=NBLK)
            for r in range(14):
                nc.sync.dma_start(
                    out=Gv_q[1:15, :, 1 + 14 * r:15 + 14 * r],
                    in_=sv_q[15 - r:29 - r, :, 1 + 14 * r:15 + 14 * r])
            nc.gpsimd.dma_start(out=Gv_q[15:16, :, 1:197], in_=sv_q[0:1, :, 1:197])
            shT_q = svb_hT[:].rearrange("p (blk w) -> p blk w", blk=NBLK)
            Gh_q = Gh[:].rearrange("p (blk w) -> p blk w", blk=NBLK)
            for c in range(14):
                eng = nc.gpsimd if c % 2 == 0 else nc.scalar
                eng.dma_start(
                    out=Gh_q[0:14, :, 14 * c:14 * (c + 1)],
                    in_=shT_q[15 - c:29 - c, :, 14 * c:14 * (c + 1)])
            return Gv, Gh

        def emit_proj(q):
            o0 = q * HB * N
            on = (HB * N) if q + 1 < NB // HB else (NTP - o0)
            for m in range(5):
                acc = pp.tile([128, on], f32, tag="projps", name="accy")
                for c in range(5):
                    nc.tensor.matmul(
                        acc[:], wp[c][:, 128 * m:128 * (m + 1)],
                        outT[c][:, o0:o0 + on],
                        start=(c == 0), stop=(c == 4))
                ysb = sp.tile([128, on], f32, tag="ysb")
                nc.vector.tensor_scalar_add(out=ysb[:], in0=acc[:],
                                            scalar1=bpr[m][:])
                nc.sync.dma_start(out=yT_d[128 * m:128 * (m + 1), o0:o0 + on],
                                  in_=ysb[:])

        quarters = [[q * HB + i for i in range(HB)] for q in range(NB // HB)]
        nextG = emit_pass1_and_G(quarters[0])
        for half in range(NB // HB):
            bs = quarters[half]
            Gv, Gh = nextG

            if half + 1 < NB // HB:
                nextG = emit_pass1_and_G(quarters[half + 1])
            if pending_pass3 is not None:
                pending_pass3()
                pending_pass3 = None
                emit_proj(half - 1)

            # pass 2: scores^T, exp, block-sums+rowsum, normalize, attn@v
            Gh_b = Gh[:].rearrange("p (blk c r) -> p blk r c", blk=NBLK, c=14)
            bhT_b = bstb_hT[:].rearrange("p (blk c r) -> p blk c r",
                                         blk=NBLK, c=14)
            for bl, b in enumerate(bs):
                t0 = b * N
                for h in range(H):
                    bv = blkv(bl, h)
                    i0 = bl * H + h
                    qT = qkT[h // 2][(h % 2) * 64:(h % 2) * 64 + 64, t0:t0 + W]
                    kTt = qkT[5 + h // 2]
                    kp = (h % 2) * 64
                    at = [ap.tile([128, W], bf16, tag="at0", name="at0"),
                          ap.tile([69, W], bf16, tag="at1", name="at1")]
                    for s, (k0, kn) in enumerate([(0, 128), (128, 69)]):
                        st = pq.tile([kn, W], f32, tag="stps")
                        nc.tensor.matmul(st[:], kTt[kp:kp + 64, t0 + k0:t0 + k0 + kn],
                                         qT, start=True, stop=False)
                        nc.tensor.matmul(st[:, 1:197], EF2v[:, k0:k0 + kn],
                                         Gv[:, bv + 1:bv + 197],
                                         start=False, stop=False)
                        nc.tensor.matmul(st[:, 1:197], EF2h[:, k0:k0 + kn],
                                         Gh_b[:, i0:i0 + 1, :, :],
                                         start=False, stop=True)
                        nc.scalar.activation(out=at[s][:], in_=st[:], func=EXP,
                                             scale=SCALE)
                    bst = p3.tile([80, W], f32, tag="sm", name="bstps")
                    nc.tensor.matmul(bst[:], E2a[:], at[0][:],
                                     start=True, stop=False)
                    nc.tensor.matmul(bst[:], E2b[:], at[1][:],
                                     start=False, stop=True)
                    rcp = ap.tile([1, W], f32, tag="rcp", name="rcp")
                    nc.vector.reciprocal_approx_fast(out=rcp[:], in_=bst[0:1, :])
                    rbs = ap.tile([80, W], f32, tag="rbs", name="rbs")
                    nc.gpsimd.partition_broadcast(rbs[:], rcp[:])
                    nc.vector.tensor_tensor(
                        out=bstb_v[:, bv:bv + W],
                        in0=bst[0:16, :], in1=rbs[64:80, :],
                        op=mybir.AluOpType.mult)
                    nc.vector.tensor_tensor(
                        out=bhT_b[0:15, i0:i0 + 1, :, :],
                        in0=bst[64:79, 1:197].rearrange("p (r c) -> p c r", r=14),
                        in1=rbs[64:79, 1:197].rearrange("p (r c) -> p c r", r=14),
                        op=mybir.AluOpType.mult)
                    av = p3.tile([64, W], f32, tag="sm", name="avps")
                    nc.tensor.matmul(av[:], vt[b][0][:, h * 64:h * 64 + 64],
                                     at[0][:], start=True, stop=False)
                    nc.tensor.matmul(av[:], vt[b][1][:, h * 64:h * 64 + 64],
                                     at[1][:], start=False, stop=True)
                    dst = outT[h // 2][(h % 2) * 64:(h % 2) * 64 + 64, t0:t0 + N]
                    nc.vector.tensor_tensor(out=dst, in0=av[:, 0:N],
                                            in1=rbs[0:64, 0:N],
                                            op=mybir.AluOpType.mult)

            # reverse scatter C2v (natural) / C2h (col-major): collapsed DMAs
            C2v_q = C2v[:].rearrange("p (blk w) -> p blk w", blk=NBLK)
            bv_q = bstb_v[:].rearrange("p (blk w) -> p blk w", blk=NBLK)
            for r in range(14):
                nc.sync.dma_start(
                    out=C2v_q[15 - r:29 - r, :, 1 + 14 * r:15 + 14 * r],
                    in_=bv_q[1:15, :, 1 + 14 * r:15 + 14 * r])
            nc.gpsimd.dma_start(out=C2v_q[0:1, :, 1:197], in_=bv_q[15:16, :, 1:197])
            bhT_q = bstb_hT[:].rearrange("p (blk w) -> p blk w", blk=NBLK)
            C2h_q = C2h[:].rearrange("p (blk w) -> p blk w", blk=NBLK)
            for c in range(14):
                nc.gpsimd.dma_start(
                    out=C2h_q[15 - c:29 - c, :, 14 * c:14 * (c + 1)],
                    in_=bhT_q[0:14, :, 14 * c:14 * (c + 1)])
            nc.gpsimd.dma_start(out=C2h_q[0:1, :, :], in_=bhT_q[14:15, :, :])

            # pass 3 (deferred one quarter so the C2 scatter wave overlaps
            # the next quarter's pass-1 PE work)
            def make_pass3(bs_):
                def emit():
                    C2h_b = C2h[:].rearrange(
                        "p (blk c r) -> p blk r c", blk=NBLK, c=14)
                    for bl, b in enumerate(bs_):
                        t0 = b * N
                        for h in range(H):
                            bv = blkv(bl, h)
                            i0 = bl * H + h
                            acc = p3.tile([64, W], f32, tag="sm", name="rpps")
                            nc.tensor.matmul(acc[:, 1:197], rpv_v[:],
                                             C2v[:, bv + 1:bv + 197],
                                             start=True, stop=False)
                            nc.tensor.matmul(acc[:, 1:197], rpv_h[:],
                                             C2h_b[:, i0:i0 + 1, :, :],
                                             start=False, stop=True)
                            dst = outT[h // 2][(h % 2) * 64:(h % 2) * 64 + 64,
                                               t0 + 1:t0 + N]
                            nc.vector.tensor_tensor(
                                out=dst, in0=dst, in1=acc[:, 1:N],
                                op=mybir.AluOpType.add)
                return emit
            pending_pass3 = make_pass3(bs)

        if pending_pass3 is not None:
            pending_pass3()
            pending_pass3 = None
        emit_proj(NB // HB - 1)

    nc.compile()
    return nc


_NC_CACHE = None


def kernel(x, w_qkv, w_proj, b_proj, rpk_v, rpk_h, rpv_v, rpv_h):
    global _NC_CACHE
    from concourse.bass_utils import run_bass_kernel_spmd

    if _NC_CACHE is None:
        _NC_CACHE = _build_nc()
    nc = _NC_CACHE

    E2, EF2v, EF2h = _consts()
    wqkvT = np.ascontiguousarray(w_qkv.T).astype(ml_dtypes.bfloat16)
    wprojT = np.ascontiguousarray(w_proj.T).astype(ml_dtypes.bfloat16)
    bproj = np.asarray(b_proj, np.float32).reshape(C, 1)
    rpkT2 = np.zeros((128, 128), ml_dtypes.bfloat16)
    rpkT2[0:64, 0:30] = rpk_v.T.astype(np.float32)
    rpkT2[0:64, 32:62] = rpk_h.T.astype(np.float32)
    rpkT2[64:128, 64:94] = rpk_v.T.astype(np.float32)
    rpkT2[64:128, 96:126] = rpk_h.T.astype(np.float32)
    rpv2 = np.concatenate([rpv_v, rpv_h], axis=0).astype(ml_dtypes.bfloat16)

    in_maps = []
    for i in range(NCORES):
        xs = np.asarray(x[i * NB:(i + 1) * NB], np.float32).reshape(NT, C)
        xT = np.zeros((C, NTP), ml_dtypes.bfloat16)
        xT[:, :NT] = xs.T.astype(ml_dtypes.bfloat16)
        in_maps.append({
            "xT": xT, "wqkvT": wqkvT, "wprojT": wprojT, "bproj": bproj,
            "rpkT2": rpkT2, "rpv2": rpv2,
            "EF2v": EF2v.astype(ml_dtypes.bfloat16),
            "EF2h": EF2h.astype(ml_dtypes.bfloat16),
            "E2": E2.astype(ml_dtypes.bfloat16),
        })

    trace = bool(os.environ.get("BASS_KERNEL_TRACE"))
    kw = {}
    if trace:
        kw = dict(trace=True, tmpdir=os.environ.get("BASS_KERNEL_TRACE_DIR") or None)
    res = run_bass_kernel_spmd(nc, in_maps, core_ids=list(range(NCORES)), **kw)
    kernel.last_result = res

    y = np.empty((B, N, C), np.float32)
    for i in range(NCORES):
        y[i * NB:(i + 1) * NB] = res.results[i]["yT"][:, :NT].T.reshape(NB, N, C)
    # cls-row rel-pos-v correction (constant across batch/query head mix)
    rep = np.tile((rpv_v[0] + rpv_h[0]).astype(np.float32), H)
    y[:, 0, :] += w_proj.astype(np.float32) @ rep
    return y


# revision 37
# speedup vs baseline: 1.1931x; 1.1931x over previous
"""AttentionSuper (AutoFormer 2D rel-pos attention) Trainium2 Bass kernel.

B=64,N=197,C=640,H=10,D=64 ; data-parallel over batch: 8 batches/core x 8 cores.
Layout (per core, feature dim on partitions):
  xT (640,1640) -> qkT (10x[128,1640] tiles, tok free), v per batch (tok part)
  S^T[k,q] per (b,h) with rel-pos bias via constant one-hot matmuls.
  Rel-pos shift gathers are SBUF->SBUF DMAs with >=28B runs:
    - row (v-table) shifts act on the natural q order (row-groups contiguous)
    - col (h-table) shifts act on a grid-transposed (col-major) copy built by
      one strided vector-engine copy; the expansion matmul reads the col-major
      store back in natural order through a strided moving AP.
  Normalization: reciprocal of the rowsum row of the block-sum matmul,
  broadcast by a ones-matmul; applied to attn@v and to the block sums before
  the reverse (rel-pos-v) scatter, so pass 3 is one matmul + one in-place add.
Host: transposes/reshapes, builds constant tables, adds the cls-row rpv fix.
"""

import os
import numpy as np
import ml_dtypes

B, N, C = 64, 197, 640
H, D = 10, 64
MRP = 14
TABLE = 30
NB = 8          # batches per core
NCORES = 8
NT = NB * N     # 1576 tokens per core
NTP = NT + 64   # padded to 1640
NP = 208        # per-(b,h) q block width, natural order
NQ = 196        # per-(b,h) q block width, col-major (grid only)
HB = 2          # batches per gather group (quarter)
SCALE = D ** -0.5

_dt = None


def _consts():
    """Constant one-hot matrices (host, numpy)."""
    r = np.arange(N - 1)
    kr = r // 14
    kc = r % 14
    # E2 (197,80): attn^T -> block sums. col 0 ones (rowsum);
    # cols 1..14 v-blocks, 15 v-cls; cols 64..77 h-blocks, 78 h-cls
    E2 = np.zeros((N, 80), np.float32)
    E2[:, 0] = 1.0
    E2[1:, :][np.arange(N - 1), 1 + kr] = 1.0
    E2[0, 15] = 1.0
    E2[1:, :][np.arange(N - 1), 64 + kc] = 1.0
    E2[0, 78] = 1.0
    # EF2v (16,197): row 0 h-cls, rows 1..14 v-shift rows, 15 v-cls
    EF2v = np.zeros((16, N), np.float32)
    EF2v[0, 0] = 1.0
    EF2v[:, 1:][1 + kr, np.arange(N - 1)] = 1.0
    EF2v[15, 0] = 1.0
    # EF2h (14,197): rows j = [kc(k)=j], k>=1
    EF2h = np.zeros((14, N), np.float32)
    EF2h[:, 1:][kc, np.arange(N - 1)] = 1.0
    return E2, EF2v, EF2h


def _build_nc():
    import concourse.bass as bass  # noqa: F401
    import concourse.mybir as mybir
    from concourse import bacc
    from concourse.tile import TileContext

    global _dt
    _dt = mybir.dt
    f32 = mybir.dt.float32
    bf16 = mybir.dt.bfloat16
    EXP = mybir.ActivationFunctionType.Exp

    nc = bacc.Bacc("TRN2", target_bir_lowering=False, debug=False,
                   num_devices=NCORES)

    xT_d = nc.dram_tensor("xT", [C, NTP], bf16, kind="ExternalInput")
    wqkvT_d = nc.dram_tensor("wqkvT", [C, 3 * C], bf16, kind="ExternalInput")
    wprojT_d = nc.dram_tensor("wprojT", [C, C], bf16, kind="ExternalInput")
    bproj_d = nc.dram_tensor("bproj", [C, 1], f32, kind="ExternalInput")
    rpkT2_d = nc.dram_tensor("rpkT2", [128, 128], bf16, kind="ExternalInput")
    rpv2_d = nc.dram_tensor("rpv2", [2 * TABLE, D], bf16, kind="ExternalInput")
    EF2v_d = nc.dram_tensor("EF2v", [16, N], bf16, kind="ExternalInput")
    EF2h_d = nc.dram_tensor("EF2h", [14, N], bf16, kind="ExternalInput")
    E2_d = nc.dram_tensor("E2", [N, 80], bf16, kind="ExternalInput")
    yT_d = nc.dram_tensor("yT", [C, NTP], f32, kind="ExternalOutput")

    W = NP
    W2 = NQ
    NBLK = HB * H         # 40 (b,h) blocks per half

    from contextlib import ExitStack
    with TileContext(nc) as tc, ExitStack() as es:
        es.enter_context(nc.allow_low_precision(reason="bf16 attention kernel"))
        cp = es.enter_context(tc.tile_pool(name="const", bufs=1))
        qp = es.enter_context(tc.tile_pool(name="qk", bufs=1))
        op = es.enter_context(tc.tile_pool(name="outacc", bufs=1))
        vp = es.enter_context(tc.tile_pool(name="v", bufs=1))
        gp = es.enter_context(tc.tile_pool(name="gather", bufs=1))
        ap = es.enter_context(tc.tile_pool(name="attn", bufs=4))
        sp = es.enter_context(tc.tile_pool(name="small", bufs=3))
        pp = es.enter_context(tc.tile_pool(name="ps", bufs=2, space="PSUM"))
        pq = es.enter_context(tc.tile_pool(name="psq", bufs=3, space="PSUM"))
        p3 = es.enter_context(tc.tile_pool(name="ps3", bufs=3, space="PSUM"))

        # ---- load constants / weights ----
        wq = [cp.tile([128, 3 * C], bf16, tag=f"wq{c}", name=f"wq{c}") for c in range(5)]
        wp = [cp.tile([128, C], bf16, tag=f"wp{c}", name=f"wp{c}") for c in range(5)]
        xT = [cp.tile([128, NTP], bf16, tag=f"xT{c}", name=f"xT{c}") for c in range(5)]
        bpr = [cp.tile([128, 1], f32, tag=f"bp{c}", name=f"bpr{c}") for c in range(5)]
        for c in range(5):
            nc.sync.dma_start(out=wq[c][:], in_=wqkvT_d[128 * c:128 * (c + 1), :])
            nc.sync.dma_start(out=wp[c][:], in_=wprojT_d[128 * c:128 * (c + 1), :])
            nc.sync.dma_start(out=xT[c][:], in_=xT_d[128 * c:128 * (c + 1), :])
            nc.sync.dma_start(out=bpr[c][:], in_=bproj_d[128 * c:128 * (c + 1), :])
        rpkT2 = cp.tile([128, 128], bf16)
        rpv_v = cp.tile([TABLE, D], bf16)
        rpv_h = cp.tile([TABLE, D], bf16)
        EF2v = cp.tile([16, N], bf16)
        EF2h = cp.tile([14, N], bf16)
        E2a = cp.tile([128, 80], bf16)
        E2b = cp.tile([69, 80], bf16)
        nc.sync.dma_start(out=rpkT2[:], in_=rpkT2_d[:])
        nc.sync.dma_start(out=rpv_v[:], in_=rpv2_d[0:TABLE, :])
        nc.sync.dma_start(out=rpv_h[:], in_=rpv2_d[TABLE:2 * TABLE, :])
        nc.sync.dma_start(out=EF2v[:], in_=EF2v_d[:])
        nc.sync.dma_start(out=EF2h[:], in_=EF2h_d[:])
        nc.sync.dma_start(out=E2a[:], in_=E2_d[0:128, :])
        nc.sync.dma_start(out=E2b[:], in_=E2_d[128:197, :])

        # per-quarter gather tiles; Gv/Gh double-buffered via gp2
        gp2 = es.enter_context(tc.tile_pool(name="g2", bufs=2))
        svb_v = gp.tile([TABLE, NBLK * W], bf16, tag="svbv", name="svbv")
        svb_hT = gp.tile([TABLE, NBLK * W2], bf16, tag="svbhT", name="svbhT")
        bstb_v = gp.tile([16, NBLK * W], bf16, tag="bstbv", name="bstbv")
        bstb_hT = gp.tile([16, NBLK * W2], bf16, tag="bstbhT", name="bstbhT")
        C2v = gp.tile([TABLE, NBLK * W], bf16, tag="C2v", name="C2v")
        C2h = gp.tile([TABLE, NBLK * W2], bf16, tag="C2h", name="C2h")
        # rows 1,29 of C2v/C2h are read by the rpv matmuls but never written
        nc.gpsimd.memset(C2v[:], 0.0)
        nc.gpsimd.memset(C2h[:], 0.0)

        def blkv(bl, h):
            return (bl * H + h) * W

        WV = NBLK * W
        WH = NBLK * W2
        APc = type(svb_v[:])

        def mkap(tile, offset, dims):
            return APc(tile[:].tensor, offset, dims)

        # ---- qk projection: qkT[o, tok] for o in 0..1279 ----
        qkT = [qp.tile([128, NTP], bf16, tag=f"qkT{m}", name=f"qkT{m}") for m in range(10)]
        chunks = [(0, 512), (512, 512), (1024, 512), (1536, NTP - 1536)]
        for m in range(10):
            for (o0, on) in chunks:
                acc = pp.tile([128, on], f32, tag="projps")
                for c in range(5):
                    nc.tensor.matmul(
                        acc[:], wq[c][:, 128 * m:128 * (m + 1)],
                        xT[c][:, o0:o0 + on],
                        start=(c == 0), stop=(c == 4))
                nc.scalar.copy(out=qkT[m][:, o0:o0 + on], in_=acc[:])

        # ---- v projection for all batches: v[b] = (tok part, 640 free) ----
        vt = [[vp.tile([128, C], bf16, tag=f"v{b}_0", name=f"v{b}_0"),
               vp.tile([69, C], bf16, tag=f"v{b}_1", name=f"v{b}_1")]
              for b in range(NB)]
        for b in range(NB):
            t0 = b * N
            for s, (ts, tn) in enumerate([(t0, 128), (t0 + 128, 69)]):
                for (o0, on) in [(0, 512), (512, 128)]:
                    acc = pp.tile([tn, on], f32, tag="projps", name="accv")
                    for c in range(5):
                        nc.tensor.matmul(
                            acc[:], xT[c][:, ts:ts + tn],
                            wq[c][:, 2 * C + o0:2 * C + o0 + on],
                            start=(c == 0), stop=(c == 4))
                    nc.scalar.copy(out=vt[b][s][:, o0:o0 + on], in_=acc[:])

        # ---- out accumulator (c part, tok free) ----
        outT = [op.tile([128, NTP], bf16, tag=f"oT{m}", name=f"oT{m}") for m in range(5)]
        for m in range(5):
            nc.gpsimd.memset(outT[m][:, NT:NTP], 0.0)

        pending_pass3 = None

        def emit_pass1_and_G(bs):
            # pass 1: combined v+h table matmul; h-part written straight to
            # col-major svb_hT, h-cls row straight into Gv row 0
            Gv = gp2.tile([16, NBLK * W], bf16, tag="Gv", name="Gv")
            Gh = gp2.tile([14, NBLK * W2], bf16, tag="Gh", name="Gh")
            shT_b = svb_hT[:].rearrange("p (blk c r) -> p blk c r",
                                        blk=NBLK, c=14)
            for bl, b in enumerate(bs):
                t0 = b * N
                for hpi in range(H // 2):
                    be, bo = blkv(bl, 2 * hpi), blkv(bl, 2 * hpi + 1)
                    s_ps = p3.tile([128, W], f32, tag="sm", name="svps")
                    nc.tensor.matmul(s_ps[:], rpkT2[:],
                                     qkT[hpi][:, t0:t0 + W],
                                     start=True, stop=True)
                    nc.scalar.copy(out=svb_v[:, be:be + W], in_=s_ps[0:30, :])
                    nc.vector.tensor_copy(out=svb_v[:, bo:bo + W],
                                          in_=s_ps[64:94, :])
                    for p0, blk in ((32, 2 * hpi), (96, 2 * hpi + 1)):
                        i = bl * H + blk
                        nc.vector.tensor_copy(
                            out=shT_b[:, i:i + 1, :, :],
                            in_=s_ps[p0:p0 + 30, 1:197].rearrange(
                                "p (r c) -> p c r", r=14))
                        nc.vector.tensor_copy(
                            out=Gv[0:1, i * W + 1:i * W + 197],
                            in_=s_ps[p0:p0 + 1, 1:197])

            # gather Gv (natural) / Gh (col-major) via SBUF->SBUF DMAs
            Gv_q = Gv[:].rearrange("p (blk w) -> p blk w", blk=NBLK)
            sv_q = svb_v[:].rearrange("p (blk w) -> p blk w", blk=NBLK)
            for r in range(14):
                nc.sync.dma_start(
                    out=Gv_q[1:15, :, 1 + 14 * r:15 + 14 * r],
                    in_=sv_q[15 - r:29 - r, :, 1 + 14 * r:15 + 14 * r])
            nc.gpsimd.dma_start(out=Gv_q[15:16, :, 1:197], in_=sv_q[0:1, :, 1:197])
            shT_q = svb_hT[:].rearrange("p (blk w) -> p blk w", blk=NBLK)
            Gh_q = Gh[:].rearrange("p (blk w) -> p blk w", blk=NBLK)
            for c in range(14):
                eng = nc.gpsimd if c % 2 == 0 else nc.scalar
                eng.dma_start(
                    out=Gh_q[0:14, :, 14 * c:14 * (c + 1)],
                    in_=shT_q[15 - c:29 - c, :, 14 * c:14 * (c + 1)])
            return Gv, Gh

        def emit_proj(q):
            o0 = q * HB * N
            on = (HB * N) if q + 1 < NB // HB else (NTP - o0)
            for m in range(5):
                acc = pp.tile([128, on], f32, tag="projps", name="accy")
                for c in range(5):
                    nc.tensor.matmul(
                        acc[:], wp[c][:, 128 * m:128 * (m + 1)],
                        outT[c][:, o0:o0 + on],
                        start=(c == 0), stop=(c == 4))
                ysb = sp.tile([128, on], f32, tag="ysb")
                nc.vector.tensor_scalar_add(out=ysb[:], in0=acc[:],
                                            scalar1=bpr[m][:])
                nc.sync.dma_start(out=yT_d[128 * m:128 * (m + 1), o0:o0 + on],
                                  in_=ysb[:])

        quarters = [[q * HB + i for i in range(HB)] for q in range(NB // HB)]
        nextG = emit_pass1_and_G(quarters[0])
        for half in range(NB // HB):
            bs = quarters[half]
            Gv, Gh = nextG

            if half + 1 < NB // HB:
                nextG = emit_pass1_and_G(quarters[half + 1])
            if pending_pass3 is not None:
                pending_pass3()
                pending_pass3 = None
                emit_proj(half - 1)

            # pass 2: scores^T, exp, block-sums+rowsum, normalize, attn@v
            Gh_b = Gh[:].rearrange("p (blk c r) -> p blk r c", blk=NBLK, c=14)
            bhT_b = bstb_hT[:].rearrange("p (blk c r) -> p blk c r",
                                         blk=NBLK, c=14)
            for bl, b in enumerate(bs):
                t0 = b * N
                for h in range(H):
                    bv = blkv(bl, h)
                    i0 = bl * H + h
                    qT = qkT[h // 2][(h % 2) * 64:(h % 2) * 64 + 64, t0:t0 + W]
                    kTt = qkT[5 + h // 2]
                    kp = (h % 2) * 64
                    at = [ap.tile([128, W], bf16, tag="at0", name="at0"),
                          ap.tile([69, W], bf16, tag="at1", name="at1")]
                    for s, (k0, kn) in enumerate([(0, 128), (128, 69)]):
                        st = pq.tile([kn, W], f32, tag="stps")
                        nc.tensor.matmul(st[:], kTt[kp:kp + 64, t0 + k0:t0 + k0 + kn],
                                         qT, start=True, stop=False)
                        nc.tensor.matmul(st[:, 1:197], EF2v[:, k0:k0 + kn],
                                         Gv[:, bv + 1:bv + 197],
                                         start=False, stop=False)
                        nc.tensor.matmul(st[:, 1:197], EF2h[:, k0:k0 + kn],
                                         Gh_b[:, i0:i0 + 1, :, :],
                                         start=False, stop=True)
                        nc.scalar.activation(out=at[s][:], in_=st[:], func=EXP,
                                             scale=SCALE)
                    bst = p3.tile([80, W], f32, tag="sm", name="bstps")
                    nc.tensor.matmul(bst[:], E2a[:], at[0][:],
                                     start=True, stop=False)
                    nc.tensor.matmul(bst[:], E2b[:], at[1][:],
                                     start=False, stop=True)
                    rcp = ap.tile([1, W], f32, tag="rcp", name="rcp")
                    nc.vector.reciprocal_approx_fast(out=rcp[:], in_=bst[0:1, :])
                    rbs = ap.tile([80, W], f32, tag="rbs", name="rbs")
                    nc.gpsimd.partition_broadcast(rbs[:], rcp[:])
                    nc.vector.tensor_tensor(
                        out=bstb_v[:, bv:bv + W],
                        in0=bst[0:16, :], in1=rbs[64:80, :],
                        op=mybir.AluOpType.mult)
                    nc.vector.tensor_tensor(
                        out=bhT_b[0:15, i0:i0 + 1, :, :],
                        in0=bst[64:79, 1:197].rearrange("p (r c) -> p c r", r=14),
                        in1=rbs[64:79, 1:197].rearrange("p (r c) -> p c r", r=14),
                        op=mybir.AluOpType.mult)
                    av = p3.tile([64, W], f32, tag="sm", name="avps")
                    nc.tensor.matmul(av[:], vt[b][0][:, h * 64:h * 64 + 64],
                                     at[0][:], start=True, stop=False)
                    nc.tensor.matmul(av[:], vt[b][1][:, h * 64:h * 64 + 64],
                                     at[1][:], start=False, stop=True)
                    dst = outT[h // 2][(h % 2) * 64:(h % 2) * 64 + 64, t0:t0 + N]
                    nc.vector.tensor_tensor(out=dst, in0=av[:, 0:N],
                                            in1=rbs[0:64, 0:N],
                                            op=mybir.AluOpType.mult)

            # reverse scatter C2v (natural) / C2h (col-major): collapsed DMAs
            C2v_q = C2v[:].rearrange("p (blk w) -> p blk w", blk=NBLK)
            bv_q = bstb_v[:].rearrange("p (blk w) -> p blk w", blk=NBLK)
            for r in range(14):
                nc.sync.dma_start(
                    out=C2v_q[15 - r:29 - r, :, 1 + 14 * r:15 + 14 * r],
                    in_=bv_q[1:15, :, 1 + 14 * r:15 + 14 * r])
            nc.gpsimd.dma_start(out=C2v_q[0:1, :, 1:197], in_=bv_q[15:16, :, 1:197])
            bhT_q = bstb_hT[:].rearrange("p (blk w) -> p blk w", blk=NBLK)
            C2h_q = C2h[:].rearrange("p (blk w) -> p blk w", blk=NBLK)
            for c in range(14):
                nc.gpsimd.dma_start(
                    out=C2h_q[15 - c:29 - c, :, 14 * c:14 * (c + 1)],
                    in_=bhT_q[0:14, :, 14 * c:14 * (c + 1)])
            nc.gpsimd.dma_start(out=C2h_q[0:1, :, :], in_=bhT_q[14:15, :, :])

            # pass 3 (deferred one quarter so the C2 scatter wave overlaps
            # the next quarter's pass-1 PE work)
            def make_pass3(bs_):
                def emit():
                    C2h_b = C2h[:].rearrange(
                        "p (blk c r) -> p blk r c", blk=NBLK, c=14)
                    for bl, b in enumerate(bs_):
                        t0 = b * N
                        for h in range(H):
                            bv = blkv(bl, h)
                            i0 = bl * H + h
                            acc = p3.tile([64, W], f32, tag="sm", name="rpps")
                            nc.tensor.matmul(acc[:, 1:197], rpv_v[:],
                                             C2v[:, bv + 1:bv + 197],
                                             start=True, stop=False)
                            nc.tensor.matmul(acc[:, 1:197], rpv_h[:],
                                             C2h_b[:, i0:i0 + 1, :, :],
                                             start=False, stop=True)
                            dst = outT[h // 2][(h % 2) * 64:(h % 2) * 64 + 64,
                                               t0 + 1:t0 + N]
                            nc.vector.tensor_tensor(
                                out=dst, in0=dst, in1=acc[:, 1:N],
                                op=mybir.AluOpType.add)
                return emit
            pending_pass3 = make_pass3(bs)

        if pending_pass3 is not None:
            pending_pass3()
            pending_pass3 = None
        emit_proj(NB // HB - 1)

    nc.compile()
    return nc


_NC_CACHE = None


def kernel(x, w_qkv, w_proj, b_proj, rpk_v, rpk_h, rpv_v, rpv_h):
    global _NC_CACHE
    from concourse.bass_utils import run_bass_kernel_spmd

    if _NC_CACHE is None:
        _NC_CACHE = _build_nc()
    nc = _NC_CACHE

    E2, EF2v, EF2h = _consts()
    wqkvT = np.ascontiguousarray(w_qkv.T).astype(ml_dtypes.bfloat16)
    wprojT = np.ascontiguousarray(w_proj.T).astype(ml_dtypes.bfloat16)
    bproj = np.asarray(b_proj, np.float32).reshape(C, 1)
    rpkT2 = np.zeros((128, 128), ml_dtypes.bfloat16)
    rpkT2[0:64, 0:30] = rpk_v.T.astype(np.float32)
    rpkT2[0:64, 32:62] = rpk_h.T.astype(np.float32)
    rpkT2[64:128, 64:94] = rpk_v.T.astype(np.float32)
    rpkT2[64:128, 96:126] = rpk_h.T.astype(np.float32)
    rpv2 = np.concatenate([rpv_v, rpv_h], axis=0).astype(ml_dtypes.bfloat16)

    in_maps = []
    for i in range(NCORES):
        xs = np.asarray(x[i * NB:(i + 1) * NB], np.float32).reshape(NT, C)
        xT = np.zeros((C, NTP), ml_dtypes.bfloat16)
        xT[:, :NT] = xs.T.astype(ml_dtypes.bfloat16)
        in_maps.append({
            "xT": xT, "wqkvT": wqkvT, "wprojT": wprojT, "bproj": bproj,
            "rpkT2": rpkT2, "rpv2": rpv2,
            "EF2v": EF2v.astype(ml_dtypes.bfloat16),
            "EF2h": EF2h.astype(ml_dtypes.bfloat16),
            "E2": E2.astype(ml_dtypes.bfloat16),
        })

    trace = bool(os.environ.get("BASS_KERNEL_TRACE"))
    kw = {}
    if trace:
        kw = dict(trace=True, tmpdir=os.environ.get("BASS_KERNEL_TRACE_DIR") or None)
    res = run_bass_kernel_spmd(nc, in_maps, core_ids=list(range(NCORES)), **kw)
    kernel.last_result = res

    y = np.empty((B, N, C), np.float32)
    for i in range(NCORES):
        y[i * NB:(i + 1) * NB] = res.results[i]["yT"][:, :NT].T.reshape(NB, N, C)
    # cls-row rel-pos-v correction (constant across batch/query head mix)
    rep = np.tile((rpv_v[0] + rpv_h[0]).astype(np.float32), H)
    y[:, 0, :] += w_proj.astype(np.float32) @ rep
    return y
